# revision 4
# baseline (speedup 1.0000x reference)
"""Luong attention on 8 TRN2 NeuronCores, data-parallel over batch.

Per core (2 batch elements):
  keysT[u,v] = (W[d,u]).T-contracted with valuesT[d,v]   (fp32 matmul)
  score[q,v] = qT[u,q].T @ keysT[u,v]                    (fp32 matmul)
  softmax rows via reduce_max(negate) + Exp activation (accum_out=rowsum)
  alignment written fp32; PE-transposed per 128x128 tile -> bf16 alignT
  context[q,d] = alignT.T @ values_bf16                  (bf16 matmul)

Host side: shard batch 2-per-core, pre-transpose query/values, gather outputs.
"""
import numpy as np

B, TQ, TV, D, UNITS = 16, 2048, 2048, 1024, 1024
NCORES = 8
BPC = B // NCORES  # batches per core

_CACHE = {}


def _build():
    import concourse.bass as bass
    import concourse.tile as tile
    from concourse import bacc, mybir
    from concourse.masks import make_identity

    f32 = mybir.dt.float32
    bf16 = mybir.dt.bfloat16
    ts = bass.ts

    nc = bacc.Bacc("TRN2", target_bir_lowering=False, debug=False)
    qT_d = nc.dram_tensor("qT", [BPC, UNITS, TQ], f32, kind="ExternalInput").ap()
    vT_d = nc.dram_tensor("vT", [BPC, D, TV], f32, kind="ExternalInput").ap()
    v_d = nc.dram_tensor("v", [BPC, TV, D], f32, kind="ExternalInput").ap()
    w_d = nc.dram_tensor("w", [D, UNITS], f32, kind="ExternalInput").ap()
    bias_d = nc.dram_tensor("bias", [UNITS], f32, kind="ExternalInput").ap()
    ctx_d = nc.dram_tensor("ctx", [BPC, TQ, D], f32, kind="ExternalOutput").ap()
    align_d = nc.dram_tensor("align", [BPC, TQ, TV], f32, kind="ExternalOutput").ap()

    DO = D // 128      # 8  (d outer)
    UO = UNITS // 128  # 8  (u outer)
    VO = TV // 128     # 16 (v outer)
    NQ = TQ // 128     # 16 q-strips
    NV = TV // 512     # 4  v-tiles of 512
    ND = D // 512      # 2  d-tiles of 512

    with tile.TileContext(nc) as tc:
        with (
            tc.tile_pool(name="consts", bufs=1) as consts,
            tc.tile_pool(name="keysT", bufs=1) as keysT_p,
            tc.tile_pool(name="vals", bufs=1) as vals_p,
            tc.tile_pool(name="vtchunk", bufs=2) as vt_p,
            tc.tile_pool(name="qt", bufs=2) as qt_p,
            tc.tile_pool(name="align", bufs=1) as align_p,
            tc.tile_pool(name="alignT", bufs=1) as alignT_p,
            tc.tile_pool(name="ctxsb", bufs=1) as ctx_p,
            tc.tile_pool(name="stats", bufs=8) as stats_p,
            tc.tile_pool(name="pssmall", bufs=2, space="PSUM") as ps_small,
            tc.tile_pool(name="psscore", bufs=1, space="PSUM") as ps_score,
            tc.tile_pool(name="psctx", bufs=1, space="PSUM") as ps_ctx,
        ):
            # --- constants ---
            w_sb = consts.tile([128, DO, UNITS], f32)
            nc.sync.dma_start(out=w_sb[:], in_=w_d.rearrange("(do di) u -> di do u", di=128))
            bias_sb = consts.tile([128, UO], f32)
            nc.sync.dma_start(out=bias_sb[:], in_=bias_d.rearrange("(uo ui) -> ui uo", ui=128))
            ident = consts.tile([128, 128], f32)
            make_identity(nc, ident[:])

            for b in range(BPC):
                # --- values (bf16, cast during DMA) for context matmul ---
                vals_sb = vals_p.tile([128, VO, D], bf16, tag="vals")
                nc.gpsimd.dma_start(
                    out=vals_sb[:], in_=v_d[b].rearrange("(vo vi) d -> vi vo d", vi=128)
                )

                # --- phase 1: keysT[u, v] (fp32) ---
                keysT_sb = keysT_p.tile([128, UO, TV], f32, tag="keysT")
                for vj in range(NV):
                    vt_c = vt_p.tile([128, DO, 512], f32, tag="vtc")
                    nc.sync.dma_start(
                        out=vt_c[:],
                        in_=vT_d[b].rearrange("(do di) v -> di do v", di=128)[
                            :, :, ts(vj, 512)
                        ],
                    )
                    for uo in range(UO):
                        ps = ps_small.tile([128, 512], f32, tag="pss")
                        for k in range(DO):
                            nc.tensor.matmul(
                                ps[:],
                                w_sb[:, k, ts(uo, 128)],
                                vt_c[:, k, :],
                                start=(k == 0),
                                stop=(k == DO - 1),
                            )
                        nc.scalar.activation(
                            out=keysT_sb[:, uo, ts(vj, 512)],
                            in_=ps[:],
                            func=mybir.ActivationFunctionType.Identity,
                            bias=bias_sb[:, uo : uo + 1],
                        )

                # --- phase 2+3: per q-strip ---
                for s in range(NQ):
                    qt_c = qt_p.tile([128, UO, 128], f32, tag="qtc")
                    nc.sync.dma_start(
                        out=qt_c[:],
                        in_=qT_d[b].rearrange("(uo ui) q -> ui uo q", ui=128)[
                            :, :, ts(s, 128)
                        ],
                    )
                    sc_ps = ps_score.tile([128, TV], f32, tag="psscore")
                    for vj in range(NV):
                        for k in range(UO):
                            nc.tensor.matmul(
                                sc_ps[:, ts(vj, 512)],
                                qt_c[:, k, :],
                                keysT_sb[:, k, ts(vj, 512)],
                                start=(k == 0),
                                stop=(k == UO - 1),
                            )
                    negmax = stats_p.tile([128, 1], f32, tag="negmax")
                    nc.vector.tensor_reduce(
                        out=negmax[:],
                        in_=sc_ps[:],
                        axis=mybir.AxisListType.X,
                        op=mybir.AluOpType.max,
                        negate=True,
                    )
                    align_t = align_p.tile([128, TV], f32, tag="align")
                    sumexp = stats_p.tile([128, 1], f32, tag="sumexp")
                    nc.scalar.activation(
                        out=align_t[:],
                        in_=sc_ps[:],
                        func=mybir.ActivationFunctionType.Exp,
                        bias=negmax[:],
                        accum_out=sumexp[:],
                    )
                    rinv = stats_p.tile([128, 1], f32, tag="rinv")
                    nc.vector.reciprocal(rinv[:], sumexp[:])
                    nc.vector.tensor_scalar_mul(align_t[:], align_t[:], rinv[:])
                    nc.sync.dma_start(out=align_d[b, ts(s, 128), :], in_=align_t[:])

                    alignT_t = alignT_p.tile([128, VO, 128], bf16, tag="alignT")
                    for vo in range(VO):
                        tr_ps = ps_small.tile([128, 128], f32, tag="pss")
                        nc.tensor.transpose(tr_ps[:], align_t[:, ts(vo, 128)], ident[:])
                        nc.scalar.copy(alignT_t[:, vo, :], tr_ps[:])

                    cx_ps = ps_ctx.tile([128, D], f32, tag="psctx")
                    for vo in range(VO):
                        for dj in range(ND):
                            nc.tensor.matmul(
                                cx_ps[:, ts(dj, 512)],
                                alignT_t[:, vo, :],
                                vals_sb[:, vo, ts(dj, 512)],
                                start=(vo == 0),
                                stop=(vo == VO - 1),
                            )
                    ctx_sb = ctx_p.tile([128, D], f32, tag="ctxsb")
                    nc.vector.tensor_copy(ctx_sb[:], cx_ps[:])
                    nc.sync.dma_start(out=ctx_d[b, ts(s, 128), :], in_=ctx_sb[:])
    nc.compile()
    return nc


def _get_nc():
    if "nc" not in _CACHE:
        _CACHE["nc"] = _build()
    return _CACHE["nc"]


def kernel(query, values, W_kernel, W_bias):
    from concourse.bass_utils import run_bass_kernel_spmd

    nc = _get_nc()
    query = np.ascontiguousarray(query, dtype=np.float32)
    values = np.ascontiguousarray(values, dtype=np.float32)
    qT = np.ascontiguousarray(query.transpose(0, 2, 1))
    vT = np.ascontiguousarray(values.transpose(0, 2, 1))
    in_maps = []
    for c in range(NCORES):
        sl = slice(c * BPC, (c + 1) * BPC)
        in_maps.append(
            {
                "qT": qT[sl],
                "vT": vT[sl],
                "v": values[sl],
                "w": np.ascontiguousarray(W_kernel, dtype=np.float32),
                "bias": np.ascontiguousarray(W_bias, dtype=np.float32),
            }
        )
    _CACHE["in_maps"] = in_maps
    res = run_bass_kernel_spmd(nc, in_maps, list(range(NCORES)))
    _CACHE["results"] = res
    context = np.concatenate([r["ctx"] for r in res.results], axis=0)
    alignment = np.concatenate([r["align"] for r in res.results], axis=0)
    return context, alignment


# revision 5
# speedup vs baseline: 2.0437x; 2.0437x over previous
"""Luong attention on 8 TRN2 NeuronCores, data-parallel over batch.

Per core (2 batch elements):
  keysT[u,v] = (W[d,u]).T-contracted with valuesT[d,v]   (fp32 matmul)
  score[q,v] = qT[u,q].T @ keysT[u,v]                    (fp32 matmul)
  softmax rows via reduce_max(negate) + Exp activation (accum_out=rowsum)
  alignment written fp32; PE-transposed per 128x128 tile -> bf16 alignT
  context[q,d] = alignT.T @ values_bf16                  (bf16 matmul)

Host side: shard batch 2-per-core, pre-transpose query/values, gather outputs.
"""
import numpy as np

B, TQ, TV, D, UNITS = 16, 2048, 2048, 1024, 1024
NCORES = 8
BPC = B // NCORES  # batches per core

_CACHE = {}


def _build():
    import concourse.bass as bass
    import concourse.tile as tile
    from concourse import bacc, mybir
    from concourse.masks import make_identity

    f32 = mybir.dt.float32
    f32r = mybir.dt.float32r
    bf16 = mybir.dt.bfloat16
    ts = bass.ts

    nc = bacc.Bacc("TRN2", target_bir_lowering=False, debug=False)
    qT_d = nc.dram_tensor("qT", [BPC, UNITS, TQ], f32, kind="ExternalInput").ap()
    vT_d = nc.dram_tensor("vT", [BPC, D, TV], f32, kind="ExternalInput").ap()
    v_d = nc.dram_tensor("v", [BPC, TV, D], f32, kind="ExternalInput").ap()
    w_d = nc.dram_tensor("w", [D, UNITS], f32, kind="ExternalInput").ap()
    bias_d = nc.dram_tensor("bias", [UNITS], f32, kind="ExternalInput").ap()
    ctx_d = nc.dram_tensor("ctx", [BPC, TQ, D], f32, kind="ExternalOutput").ap()
    align_d = nc.dram_tensor("align", [BPC, TQ, TV], f32, kind="ExternalOutput").ap()

    DO = D // 128      # 8  (d outer)
    UO = UNITS // 128  # 8  (u outer)
    VO = TV // 128     # 16 (v outer)
    NQ = TQ // 128     # 16 q-strips
    NV = TV // 512     # 4  v-tiles of 512
    ND = D // 512      # 2  d-tiles of 512

    with tile.TileContext(nc) as tc:
        with (
            tc.tile_pool(name="consts", bufs=1) as consts,
            tc.tile_pool(name="keysT", bufs=1) as keysT_p,
            tc.tile_pool(name="vals", bufs=1) as vals_p,
            tc.tile_pool(name="vtchunk", bufs=2) as vt_p,
            tc.tile_pool(name="qt", bufs=2) as qt_p,
            tc.tile_pool(name="align", bufs=1) as align_p,
            tc.tile_pool(name="alignT", bufs=1) as alignT_p,
            tc.tile_pool(name="ctxsb", bufs=1) as ctx_p,
            tc.tile_pool(name="stats", bufs=8) as stats_p,
            tc.tile_pool(name="pssmall", bufs=2, space="PSUM") as ps_small,
            tc.tile_pool(name="psscore", bufs=1, space="PSUM") as ps_score,
            tc.tile_pool(name="psctx", bufs=1, space="PSUM") as ps_ctx,
        ):
            # --- constants ---
            w_sb = consts.tile([128, DO, UNITS], f32r)
            nc.gpsimd.dma_start(out=w_sb[:], in_=w_d.rearrange("(do di) u -> di do u", di=128))
            bias_sb = consts.tile([128, UO], f32)
            nc.sync.dma_start(out=bias_sb[:], in_=bias_d.rearrange("(uo ui) -> ui uo", ui=128))
            ident = consts.tile([128, 128], f32)
            make_identity(nc, ident[:])

            for b in range(BPC):
                # --- values (bf16, cast during DMA) for context matmul ---
                vals_sb = vals_p.tile([128, VO, D], bf16, tag="vals")
                nc.gpsimd.dma_start(
                    out=vals_sb[:], in_=v_d[b].rearrange("(vo vi) d -> vi vo d", vi=128)
                )

                # --- phase 1: keysT[u, v] (fp32) ---
                keysT_sb = keysT_p.tile([128, UO, TV], f32r, tag="keysT")
                for vj in range(NV):
                    vt_c = vt_p.tile([128, DO, 512], f32r, tag="vtc")
                    nc.gpsimd.dma_start(
                        out=vt_c[:],
                        in_=vT_d[b].rearrange("(do di) v -> di do v", di=128)[
                            :, :, ts(vj, 512)
                        ],
                    )
                    for uo in range(UO):
                        ps = ps_small.tile([128, 512], f32, tag="pss")
                        for k in range(DO):
                            nc.tensor.matmul(
                                ps[:],
                                w_sb[:, k, ts(uo, 128)],
                                vt_c[:, k, :],
                                start=(k == 0),
                                stop=(k == DO - 1),
                            )
                        nc.scalar.activation(
                            out=keysT_sb[:, uo, ts(vj, 512)],
                            in_=ps[:],
                            func=mybir.ActivationFunctionType.Identity,
                            bias=bias_sb[:, uo : uo + 1],
                        )

                # --- phase 2+3: per q-strip ---
                for s in range(NQ):
                    qt_c = qt_p.tile([128, UO, 128], f32r, tag="qtc")
                    nc.gpsimd.dma_start(
                        out=qt_c[:],
                        in_=qT_d[b].rearrange("(uo ui) q -> ui uo q", ui=128)[
                            :, :, ts(s, 128)
                        ],
                    )
                    sc_ps = ps_score.tile([128, TV], f32, tag="psscore")
                    for vj in range(NV):
                        for k in range(UO):
                            nc.tensor.matmul(
                                sc_ps[:, ts(vj, 512)],
                                qt_c[:, k, :],
                                keysT_sb[:, k, ts(vj, 512)],
                                start=(k == 0),
                                stop=(k == UO - 1),
                            )
                    negmax = stats_p.tile([128, 1], f32, tag="negmax")
                    nc.vector.tensor_reduce(
                        out=negmax[:],
                        in_=sc_ps[:],
                        axis=mybir.AxisListType.X,
                        op=mybir.AluOpType.max,
                        negate=True,
                    )
                    align_t = align_p.tile([128, TV], f32, tag="align")
                    sumexp = stats_p.tile([128, 1], f32, tag="sumexp")
                    nc.scalar.activation(
                        out=align_t[:],
                        in_=sc_ps[:],
                        func=mybir.ActivationFunctionType.Exp,
                        bias=negmax[:],
                        accum_out=sumexp[:],
                    )
                    rinv = stats_p.tile([128, 1], f32, tag="rinv")
                    nc.vector.reciprocal(rinv[:], sumexp[:])
                    nc.vector.tensor_scalar_mul(align_t[:], align_t[:], rinv[:])
                    nc.sync.dma_start(out=align_d[b, ts(s, 128), :], in_=align_t[:])

                    alignT_t = alignT_p.tile([128, VO, 128], bf16, tag="alignT")
                    for vo in range(VO):
                        tr_ps = ps_small.tile([128, 128], f32, tag="pss")
                        nc.tensor.transpose(tr_ps[:], align_t[:, ts(vo, 128)], ident[:])
                        nc.scalar.copy(alignT_t[:, vo, :], tr_ps[:])

                    cx_ps = ps_ctx.tile([128, D], f32, tag="psctx")
                    for vo in range(VO):
                        for dj in range(ND):
                            nc.tensor.matmul(
                                cx_ps[:, ts(dj, 512)],
                                alignT_t[:, vo, :],
                                vals_sb[:, vo, ts(dj, 512)],
                                start=(vo == 0),
                                stop=(vo == VO - 1),
                            )
                    ctx_sb = ctx_p.tile([128, D], f32, tag="ctxsb")
                    nc.vector.tensor_copy(ctx_sb[:], cx_ps[:])
                    nc.sync.dma_start(out=ctx_d[b, ts(s, 128), :], in_=ctx_sb[:])
    nc.compile()
    return nc


def _get_nc():
    if "nc" not in _CACHE:
        _CACHE["nc"] = _build()
    return _CACHE["nc"]


def kernel(query, values, W_kernel, W_bias):
    from concourse.bass_utils import run_bass_kernel_spmd

    nc = _get_nc()
    query = np.ascontiguousarray(query, dtype=np.float32)
    values = np.ascontiguousarray(values, dtype=np.float32)
    qT = np.ascontiguousarray(query.transpose(0, 2, 1))
    vT = np.ascontiguousarray(values.transpose(0, 2, 1))
    in_maps = []
    for c in range(NCORES):
        sl = slice(c * BPC, (c + 1) * BPC)
        in_maps.append(
            {
                "qT": qT[sl],
                "vT": vT[sl],
                "v": values[sl],
                "w": np.ascontiguousarray(W_kernel, dtype=np.float32),
                "bias": np.ascontiguousarray(W_bias, dtype=np.float32),
            }
        )
    _CACHE["in_maps"] = in_maps
    res = run_bass_kernel_spmd(nc, in_maps, list(range(NCORES)))
    _CACHE["results"] = res
    context = np.concatenate([r["ctx"] for r in res.results], axis=0)
    alignment = np.concatenate([r["align"] for r in res.results], axis=0)
    return context, alignment
